# revision 30
# baseline (speedup 1.0000x reference)
"""Causal GQA attention (B=2, H=32, KVH=8, N=2048, D=128) on 8 trn2 cores.

Sharding: 64 (batch, q-head) problems; core c gets q-heads [4c, 4c+4) for both
batches (8 independent attention problems per core).  GQA repeat is
`(r kvh)` ordering, so q-head h uses kv-head h % 8 — each per-core q-head is
paired 1:1 with the kv head it needs; no cross-core communication.

Per-core kernel layout ("S-transposed" flash-style, no online softmax needed
since rows are bounded: exp(S*scale) computed without max subtraction):
  - host ships Q^T, K^T as [d=128, n=2048] fp16 tiles (d on partitions),
    V as [j%128 partitions, jblock, d] fp16 augmented with a ones column.
  - S^T[j,i] blocks [128, 512] = matmul(lhsT=K^T block, rhs=Q^T group) in PSUM
  - exp via ScalarE over 3-bank-wide PSUM chunks -> P^T fp16 in SBUF
  - causal mask applied POST-exp on DVE: the 4 triangular 128x128 sub-blocks
    of each diagonal group are multiplied in-place by a 0/1 lower-triangle
    (keeps the mask off the TensorE critical path; raw unmasked scores are
    bounded so fp16 exp cannot overflow)
  - PV: matmul(lhsT=P^T 128-col chunk, rhs=[V | 1]) accumulated over j blocks;
    output column 128 is the softmax denominator (rowsum)
  - finalize: batched reciprocal of rowsums, per-partition scale, DMA out
    fp32 in device-native [p, ic, d] order (host reshapes to [n, d])
"""

import sys

sys.path.insert(0, "/opt/trn_rl_repo")

import numpy as np

import concourse.bass as bass
import concourse.mybir as mybir
from concourse import bacc
import concourse.tile as tile
from concourse.bass_utils import run_bass_kernel_spmd

P = 128
NSEQ = 2048
D = 128
NH = 8          # (batch, q-head) problems per core
NG = 4          # query groups per head
GI = 512        # query rows per group
NJB = 16        # 128-wide key blocks per head
SCALE = 1.0 / np.sqrt(128.0)

F16 = mybir.dt.float16
F32 = mybir.dt.float32
PRIO_OFF = 25   # pull S-production (QK matmuls + exp) just ahead of the
                # previous chunk's PV matmuls; large values defer PVs too far
                # (in-order PE queue then stalls on S-tile waits mid-kernel
                # and drains a PV backlog after the last exp)

_NC_CACHE = {}


def build_nc(trace_scopes=False):
    nc = bacc.Bacc("TRN2", target_bir_lowering=False, debug=False, num_devices=8)

    # per-head packed input: [qT (2048) | kT (2048) | vaug (16*129)] per partition
    W_IN = 2 * NSEQ + NJB * (D + 1)
    inp_d = nc.dram_tensor("inp", [NH, P, W_IN], F16, kind="ExternalInput").ap()
    consts_d = nc.dram_tensor("consts", [P, P], F16, kind="ExternalInput").ap()
    # output in device-native layout: [head, group, partition, i-chunk, d];
    # row (g*512 + ic*128 + p) of the logical [2048, 128] output lives at
    # o_d[h, g, p, ic, :].  Host reassembles (free), device DMA is contiguous.
    o_d = nc.dram_tensor("o", [NH, NG, P, 4, D], F32, kind="ExternalOutput").ap()

    with tile.TileContext(nc) as tc:
        with (
            tc.tile_pool(name="cst", bufs=1) as cpool,
            tc.tile_pool(name="inp", bufs=4) as inpool,
            tc.tile_pool(name="pt", bufs=8) as ppool,
            tc.tile_pool(name="fin", bufs=6) as finpool,
            tc.tile_pool(name="spsum", bufs=2, space="PSUM") as spool,
            tc.tile_pool(name="opsum", bufs=1, space="PSUM") as opool,
        ):
            cst = cpool.tile([P, P], F16)
            # btri01: [128, 128] 0/1 mask, 0 where p > i (masked future positions
            # within a diagonal 128x128 block of S^T), 1 elsewhere; its
            # dma_start is emitted after head 0's first input piece (trigger
            # slots are serial and the mask isn't needed until the first
            # diagonal exp)
            btri01 = cst[:, 0:P]

            # HAM warm-up: the PE clock-gate starts at 4/8 (1.2 GHz) and
            # only releases after ~3.4us of sustained matmul activity.  Run
            # throwaway matmuls during the initial DMA window so the real
            # pipeline starts at full clock.  They write the first group's
            # O0 subtile, which the first real PV (start=True) overwrites.
            warm = cpool.tile([P, D + 1], F16)
            wO = opool.tile([P, 2, D + 1], F32, tag="O0")
            nc.vector.memset(warm[:], 0.0)
            for _ in range(34):
                nc.tensor.matmul(
                    wO[:, 0, :], warm[:, 0:P], warm[:],
                    start=True, stop=True,
                )

            for h in range(NH):
                hin = inpool.tile([P, W_IN], F16, tag="hin")
                if h == 0:
                    # groups run g3..g0; each dma_start costs ~600ns of
                    # serial trigger time on the Sync sequencer, so use few
                    # pieces: [qT g3 | kT blocks 0-2] is one contiguous
                    # range and covers the first three S matmuls
                    cuts = [
                        (3 * GI, NSEQ + 3 * P),                  # qT g3 + kT blocks 0-2
                        (NSEQ + 3 * P, NSEQ + 8 * P),            # kT blocks 3-7
                        (NSEQ + 8 * P, 2 * NSEQ),                # kT blocks 8-15
                        (2 * NSEQ, W_IN),                        # va blocks 0-15
                        (0, 3 * GI),                             # qT groups 2,1,0
                    ]
                    # only the critical piece + consts ride the SP queue: the
                    # engine-start barrier is processed behind the SP's
                    # initial trigger burst, so every extra SP DMACopy delays
                    # ALL compute engines by ~600ns.  Bulk input goes through
                    # the otherwise-idle GpSimd software DGE.
                    with tc.high_priority(offset=None):
                        a, b = cuts[0]
                        nc.sync.dma_start(hin[:, a:b], inp_d[h, :, a:b])
                        nc.sync.dma_start(cst[:], consts_d)
                        for a, b in cuts[1:4]:
                            nc.gpsimd.dma_start(hin[:, a:b], inp_d[h, :, a:b])
                    for a, b in cuts[4:]:
                        nc.gpsimd.dma_start(hin[:, a:b], inp_d[h, :, a:b])
                else:
                    nc.gpsimd.dma_start(hin[:], inp_d[h])
                qT = hin[:, 0:NSEQ]
                kT = hin[:, NSEQ : 2 * NSEQ]
                va = hin[:, 2 * NSEQ :].rearrange("p (a b) -> p a b", b=D + 1)

                # reversed group order: each group's long diagonal exp is
                # overlapped by the NEXT group's dense S matmuls (g0 is
                # diag-only, so forward order starves the PE at every
                # head transition)
                for g in reversed(range(NG)):
                    O0 = opool.tile([P, 2, D + 1], F32, tag="O0")
                    O1 = opool.tile([P, 2, D + 1], F32, tag="O1")
                    otiles = [(O0, 0), (O0, 1), (O1, 0), (O1, 1)]

                    def pv(Pf, off, jb, ic, g=g, otiles=otiles):
                        ot, sub = otiles[ic]
                        # O0/O1 each hold two 129-wide subtiles in one PSUM
                        # bank: one start (first write) and one stop (last
                        # write) per bank
                        nc.tensor.matmul(
                            ot[:, sub, :],
                            Pf[:, off : off + P],
                            va[:, jb, :],
                            start=(jb == 0 and ic % 2 == 0),
                            stop=(jb == 4 * g + ic and ic % 2 == 1),
                        )

                    # dense key blocks (jb < 4g), 3 per PSUM tile
                    for c0 in range(0, 4 * g, 3):
                        chunk = list(range(c0, min(c0 + 3, 4 * g)))
                        ln = len(chunk)
                        with tc.high_priority(offset=PRIO_OFF):
                            S = spool.tile([P, 3, GI], F32, tag="S")
                            for s, jb in enumerate(chunk):
                                nc.tensor.matmul(
                                    S[:, s, :],
                                    kT[:, jb * P : (jb + 1) * P],
                                    qT[:, g * GI : (g + 1) * GI],
                                    start=True,
                                    stop=True,
                                )

                            Pt = ppool.tile([P, 3, GI], F16, tag="P")
                            nc.scalar.activation(
                                Pt[:, 0:ln, :],
                                S[:, 0:ln, :],
                                mybir.ActivationFunctionType.Exp,
                                scale=float(SCALE),
                            )
                        Pf = Pt[:].rearrange("p a b -> p (a b)")
                        for s, jb in enumerate(chunk):
                            for ic in range(4):
                                pv(Pf, s * GI + ic * P, jb, ic)

                    # diagonal group (jb = 4g+r, r=0..3): only the unmasked
                    # suffix of each block is computed, packed contiguously:
                    #   bank0: r0 (512) | bank1: r1 (384) + r3 (128) |
                    #   bank2: r2 (256)  -> one 1280-wide exp
                    with tc.high_priority(offset=PRIO_OFF):
                        S = spool.tile([P, 3, GI], F32, tag="S")
                        Sf = S[:].rearrange("p a b -> p (a b)")
                        roff = {0: 0, 1: GI, 3: GI + 384, 2: 2 * GI}
                        rw = {0: 512, 1: 384, 3: 128, 2: 256}
                        # per bank: one accumulation group (one start, one stop)
                        bank_rs = [(0,), (1, 3), (2,)]
                        for rs in bank_rs:
                            for pos, r in enumerate(rs):
                                jb = 4 * g + r
                                nc.tensor.matmul(
                                    Sf[:, roff[r] : roff[r] + rw[r]],
                                    kT[:, jb * P : (jb + 1) * P],
                                    qT[:, g * GI + r * P : (g + 1) * GI],
                                    start=(pos == 0),
                                    stop=(pos == len(rs) - 1),
                                )
                        Pt = ppool.tile([P, 3, GI], F16, tag="P")
                        Pf = Pt[:].rearrange("p a b -> p (a b)")
                        nc.scalar.activation(
                            Pf[:, 0:1280],
                            Sf[:, 0:1280],
                            mybir.ActivationFunctionType.Exp,
                            scale=float(SCALE),
                        )
                    # zero the masked (future) half of each triangular
                    # sub-block: in-place fp16 multiply by 0/1 mask.  The
                    # triangles sit at cols {0, 512} (stride 512: r0, r1) and
                    # {896, 1024} (contiguous: r3, r2), so two strided ops
                    # cover all four.  These gate the diagonal PVs, so they
                    # must beat the previous group's finalize in the DVE queue.
                    with tc.high_priority(offset=160):
                        bt2 = btri01.rearrange("p (a c) -> p a c", a=1).broadcast_to(
                            [P, 2, P]
                        )
                        mA = Pt[:, 0:2, 0:P]          # cols 0-127 and 512-639
                        nc.vector.tensor_mul(mA, mA, bt2)
                        mB = Pf[:, 896:1152].rearrange("p (a b) -> p a b", b=P)
                        nc.vector.tensor_mul(mB, mB, bt2)
                    # ic-major order: O0's bank (chunks 0,1) is complete after
                    # the ic==1 iteration, so its finalize overlaps chunks
                    # 2-3's PV matmuls and frees the bank for the next group
                    # that much earlier
                    osb = finpool.tile([P, 4, D], F32, tag="osb")
                    rec = finpool.tile([P, 4], F32, tag="rec")
                    for ic in range(4):
                        for r in range(ic + 1):
                            pv(Pf, roff[r] + (ic - r) * P, 4 * g + r, ic)
                        if ic == 1 or ic == 3:
                            ot = O0 if ic == 1 else O1
                            c0 = ic - 1
                            with tc.high_priority(offset=50):
                                nc.vector.reciprocal(
                                    rec[:, c0 : c0 + 2], ot[:, :, D : D + 1]
                                )
                                nc.vector.tensor_mul(
                                    osb[:, c0 : c0 + 2, :],
                                    ot[:, :, 0:D],
                                    rec[:, c0 : c0 + 2].broadcast_to([P, 2, D]),
                                )
                    with tc.high_priority(offset=50):
                        nc.sync.dma_start(o_d[h, g], osb[:])
    nc.compile()
    return nc


def _get_nc():
    if "nc" not in _NC_CACHE:
        _NC_CACHE["nc"] = build_nc()
    return _NC_CACHE["nc"]


def make_consts():
    pp = np.arange(P)[:, None]
    ii = np.arange(P)[None, :]
    return np.where(pp > ii, np.float16(0.0), np.float16(1.0)).astype(np.float16)


def make_in_maps(q, k, v):
    """Shard full inputs into 8 per-core input maps (host-side layout prep)."""
    consts = make_consts()
    W_IN = 2 * NSEQ + NJB * (D + 1)
    in_maps = []
    for c in range(8):
        inp = np.empty((NH, P, W_IN), dtype=np.float16)
        i = 0
        for b in range(2):
            for qh in range(4 * c, 4 * c + 4):
                kvh = qh % 8
                inp[i, :, 0:NSEQ] = q[b, qh].T
                inp[i, :, NSEQ : 2 * NSEQ] = k[b, kvh].T
                va = inp[i, :, 2 * NSEQ :].reshape(P, NJB, D + 1)
                # v[b,kvh]: [2048, 128] -> [jb, p, d] -> [p, jb, d]
                va[:, :, :D] = v[b, kvh].reshape(NJB, P, D).transpose(1, 0, 2)
                va[:, :, D] = 1.0
                i += 1
        in_maps.append({"inp": inp, "consts": consts})
    return in_maps


def assemble_output(results):
    out = np.empty((2, 32, NSEQ, D), dtype=np.float32)
    for c in range(8):
        o = results[c]["o"]  # [NH, NG, P, 4, D]; row g*512 + ic*128 + p
        i = 0
        for b in range(2):
            for qh in range(4 * c, 4 * c + 4):
                out[b, qh] = o[i].transpose(0, 2, 1, 3).reshape(NSEQ, D)
                i += 1
    return out


def _install_ntff_hook():
    """The agent image's antenv lacks axon_hooks; inject a shim so
    run_bass_kernel_spmd(trace=True) can reach the NTFF profiler in
    libaxon_pjrt.so. Only needed for profiling runs."""
    import types

    if "antenv.axon_hooks" in sys.modules:
        return
    mod = types.ModuleType("antenv.axon_hooks")
    _h = [None]
    mod.set_axon_ntff_profile_hook = lambda h: _h.__setitem__(0, h)
    mod.get_axon_ntff_profile_hook = lambda: _h[0]
    sys.modules["antenv.axon_hooks"] = mod
    import antenv

    antenv.axon_hooks = mod
    if "/root/.axon_site" not in sys.path:
        sys.path.insert(0, "/root/.axon_site")
    from trn_agent_boot.trn_boot import _ntff_profile_via_ctypes

    hook = _ntff_profile_via_ctypes("/opt/axon/libaxon_pjrt.so")
    if hook is not None:
        mod.set_axon_ntff_profile_hook(hook)

    # avoid S3-ish artifact upload in this container
    import concourse.bass_utils as bu

    bu.upload_artifacts = lambda tmpdir: tmpdir


def kernel(q, k, v, _trace=False, _trace_kwargs=None):
    q = np.asarray(q, dtype=np.float32)
    k = np.asarray(k, dtype=np.float32)
    v = np.asarray(v, dtype=np.float32)
    assert q.shape == (2, 32, NSEQ, D), q.shape
    assert k.shape == (2, 8, NSEQ, D), k.shape
    assert v.shape == (2, 8, NSEQ, D), v.shape

    nc = _get_nc()
    in_maps = make_in_maps(q, k, v)
    kwargs = {}
    if _trace:
        _install_ntff_hook()
        kwargs["trace"] = True
        kwargs.update(_trace_kwargs or {})
    res = run_bass_kernel_spmd(nc, in_maps, core_ids=list(range(8)), **kwargs)
    out = assemble_output(res.results)
    if _trace:
        return out, res
    return out
